# revision 15
# baseline (speedup 1.0000x reference)
"""TreeLSTM-style DERNN kernel for Trainium2 (Bass/Tile), 8-core data-parallel.

Strategy (v2)
-------------
- Shard the 512 trees across 8 cores (64 trees/core); each tree is a
  complete binary tree of 127 nodes, processed level-synchronously
  (leaves -> root).
- Host-side prep does the embedding gather + transpose: x arrives as
  fp8 [feature, node] streams in level-major order, with each level laid
  out [left-children | right-children] so pair reductions are contiguous
  adds and the f-gate can reuse the parent's x stream for both halves.
- All x-side projections run as fp8 e4m3 DoubleRow matmuls (2 K-tiles
  per pass, 0.5 cyc/row). The dep-type terms are folded into the K
  remainder tile (rows 44:54 = one-hot / pair-sum one-hot), the leaf
  iu constant into row 54, and all biases into the host-prepped weight
  tiles, so there are no separate dep matmuls and no ACT biases.
- U·h terms stay bf16 (fp8 h fails accuracy) and accumulate into the
  same PSUM region as the x projections; gates activate directly from
  PSUM. PSUM: 2 pools x 2 bufs x 2 banks = all 8 banks, giving depth-2
  chunk pipelining so the PE never drains (pstate stays at 2.4 GHz).
"""

import os
import sys

import numpy as np

for _p in ("/opt/trn_rl_repo", "/root/.axon_site/_ro/trn_rl_repo"):
    if _p not in sys.path and os.path.isdir(_p):
        sys.path.append(_p)

B, N, H, E, V, Q = 512, 127, 256, 300, 50000, 10
NCORES = 8
BT = B // NCORES          # trees per core
NN = BT * 127             # nodes per core
CN = BT * 126             # child nodes per core (levels 0..5)
LS = [BT * (64 >> lv) for lv in range(7)]    # level sizes, lv0 = leaves
NOFF = [0]
for _lv in range(7):
    NOFF.append(NOFF[-1] + LS[_lv])

PCH = 256    # parent chunk (internal levels)
LCH = 512    # leaf chunk

USE_BCAST = True   # single f-x matmul with broadcast-halves AP


def _order():
    """Level-major node order; within each level [left kids | right kids]
    of the previous (parent) level's order. Returns flat node ids."""
    t = np.arange(BT) * 127
    ords = {6: t.copy()}                     # roots
    for lv in range(5, -1, -1):
        par = ords[lv + 1]
        tt = par // 127
        n = par % 127
        left = tt * 127 + 2 * n + 1
        right = tt * 127 + 2 * n + 2
        ords[lv] = np.concatenate([left, right])
    return np.concatenate([ords[lv] for lv in range(7)])


PERM = _order()


def build_nc():
    import concourse.bacc as bacc
    import concourse.bass as bass  # noqa: F401
    import concourse.mybir as mybir
    import concourse.tile as tile

    f32 = mybir.dt.float32
    bf16 = mybir.dt.bfloat16
    f8 = mybir.dt.float8e4
    AF = mybir.ActivationFunctionType
    DR = mybir.MatmulPerfMode.DoubleRow

    nc = bacc.Bacc("TRN2", target_bir_lowering=False, debug=False,
                   num_devices=NCORES)

    xp_d = [nc.declare_dram_parameter(f"xp{lv}", [128, 2 * LS[lv]], f8,
                                      isOutput=False) for lv in range(7)]
    x2iu_d = [nc.declare_dram_parameter(f"x2iu{lv}", [66, LS[lv]], f8,
                                        isOutput=False) for lv in range(7)]
    x2f_d = [None] + [nc.declare_dram_parameter(
        f"x2f{lv}", [64, LS[lv - 1]], f8, isOutput=False)
        for lv in range(1, 7)]
    w_d = nc.declare_dram_parameter("wk", [128, 2 * 768], f8, isOutput=False)
    w2iu_d = nc.declare_dram_parameter("w2iu", [66, 512], f8,
                                       isOutput=False)
    w2f_d = nc.declare_dram_parameter("w2f", [64, 256], f8,
                                      isOutput=False)
    u_d = nc.declare_dram_parameter("uk", [128, 2 * 768], bf16,
                                    isOutput=False)
    out_d = nc.declare_dram_parameter("out", [128, 2 * BT], bf16,
                                      isOutput=True)

    with tile.TileContext(nc) as tc:
        with (
            tc.tile_pool(name="const", bufs=1) as const,
            tc.tile_pool(name="pa", bufs=2, space="PSUM") as pa,
            tc.tile_pool(name="pb", bufs=2, space="PSUM") as pb,
            tc.tile_pool(name="work", bufs=3) as work,
        ):
            def load(dram, shape, dtype, eng=None):
                t = const.tile(shape, dtype, name=f"ld_{dram.name}")
                (eng or nc.sync).dma_start(out=t[:], in_=dram.ap())
                return t

            w_sb = load(w_d, [128, 2 * 768], f8)
            w2iu_sb = load(w2iu_d, [66, 512], f8)
            w2f_sb = load(w2f_d, [64, 256], f8)
            u_sb = load(u_d, [128, 2 * 768], bf16)

            def load2(dram, shape, dtype, nsplit):
                """Load a [P, k*L] k-tiled tensor in nsplit column slices
                so compute can start before the whole tensor arrives."""
                t = const.tile(shape, dtype, name=f"ld_{dram.name}")
                P_, tot = shape
                L = tot // 2
                step = L // nsplit
                tv = t[:].rearrange("p (k n) -> p k n", k=2)
                dv = dram.ap().rearrange("p (k n) -> p k n", k=2)
                for i, s0 in enumerate(range(0, L, step)):
                    e = nc.sync if i % 2 == 0 else nc.gpsimd
                    e.dma_start(out=tv[:, :, s0:s0 + step],
                                in_=dv[:, :, s0:s0 + step])
                return t

            def load1(dram, shape, dtype, nsplit):
                t = const.tile(shape, dtype, name=f"ld_{dram.name}")
                P_, L = shape
                step = L // nsplit
                for i, s0 in enumerate(range(0, L, step)):
                    e = nc.scalar if i % 2 == 0 else nc.gpsimd
                    e.dma_start(out=t[:, s0:s0 + step],
                                in_=dram.ap()[:, s0:s0 + step])
                return t

            xp_sb = [None] * 7
            x2iu_sb = [None] * 7
            xp_sb[0] = load2(xp_d[0], [128, 2 * LS[0]], f8, 8)
            x2iu_sb[0] = load1(x2iu_d[0], [66, LS[0]], f8, 8)
            for lv in range(1, 7):
                xp_sb[lv] = load(xp_d[lv], [128, 2 * LS[lv]], f8,
                                 nc.gpsimd if lv <= 3 else nc.sync)
                x2iu_sb[lv] = load(x2iu_d[lv], [66, LS[lv]], f8, nc.scalar)
            x2f_sb = [None] + [load(x2f_d[lv], [64, LS[lv - 1]], f8,
                                    nc.scalar if lv <= 2 else nc.gpsimd)
                               for lv in range(1, 7)]

            h_sb = [const.tile([128, 2 * LS[lv]], bf16, name=f"h{lv}")
                    for lv in range(7)]
            hs_sb = [None] + [const.tile([128, 2 * LS[lv]], bf16,
                                         name=f"hs{lv}")
                              for lv in range(1, 7)]
            fs_sb = [None] + [const.tile([128, 2 * LS[lv]], bf16,
                                         name=f"fs{lv}")
                              for lv in range(1, 7)]

            # k-tile views
            wv = w_sb[:].rearrange("p (k m) -> p k m", k=2)       # [128,2,768]
            uv = u_sb[:].rearrange("p (k m) -> p k m", k=2)       # [128,2,768]
            xpv = [xp_sb[lv][:].rearrange("p (k n) -> p k n", k=2)
                   for lv in range(7)]

            def mm(o, lhsT, rhs, start, stop, dr=False):
                nc.tensor.matmul(o, lhsT, rhs, start=start, stop=stop,
                                 perf_mode=DR if dr else None)

            # ---------------- leaves (lv 0) ----------------
            L0 = LS[0]
            for p0 in range(0, L0, LCH):
                cw = min(LCH, L0 - p0)
                psI = pa.tile([128, 1024], f32, tag="psA")
                psU = pb.tile([128, 1024], f32, tag="psB")
                si = work.tile([128, 1024], bf16, tag="siL")
                tu = work.tile([128, 1024], bf16, tag="tuL")
                for m in range(4):
                    ps = psI if m < 2 else psU
                    o = ps[:, (m % 2) * 512:(m % 2) * 512 + cw]
                    mc = slice(256 + m * 128, 256 + (m + 1) * 128)
                    mm(o, wv[:, :, mc], xpv[0][:, :, p0:p0 + cw],
                       start=True, stop=False, dr=True)
                    mm(o, w2iu_sb[:, m * 128:(m + 1) * 128],
                       x2iu_sb[0][:, p0:p0 + cw],
                       start=False, stop=True)
                    s = slice((m % 2) * 512, (m % 2) * 512 + cw)
                    if m == 1:
                        nc.scalar.activation(si[:, 0:512], psI[:, 0:512],
                                             AF.Sigmoid)
                        nc.scalar.activation(si[:, 512:1024],
                                             psI[:, 512:1024], AF.Sigmoid)
                for bk in range(2):
                    s = slice(bk * 512, bk * 512 + cw)
                    nc.scalar.activation(tu[:, s], psU[:, s], AF.Tanh)
                g = work.tile([128, 1024], bf16, tag="gL")
                nc.vector.tensor_mul(g[:, :], si[:, :], tu[:, :])
                gvw = g[:].rearrange("p (m c) -> p m c", m=2)[:, :, 0:cw]
                hovw = h_sb[0][:].rearrange("p (m c) -> p m c", m=2)[
                    :, :, p0:p0 + cw]
                nc.scalar.activation(hovw, gvw, AF.Tanh)

            # ---------------- internal levels ----------------
            for lv in range(1, 7):
                Lp, Lc = LS[lv], LS[lv - 1]
                hp = h_sb[lv - 1][:].rearrange("p (m c) -> p m c", m=2)
                hsv = hs_sb[lv][:].rearrange("p (m c) -> p m c", m=2)
                fsv = fs_sb[lv][:].rearrange("p (m c) -> p m c", m=2)
                hcv = h_sb[lv][:].rearrange("p (m c) -> p m c", m=2)

                # h_sum = h_left + h_right (contiguous halves)
                nc.vector.tensor_add(hsv[:, :, :],
                                     hp[:, :, 0:Lp], hp[:, :, Lp:Lc])

                # --- f gates, parent chunks (left+right kids in one psum) ---
                for p0 in range(0, Lp, PCH):
                    pw = min(PCH, Lp - p0)
                    psF = pb.tile([128, 1024], f32, tag="psB")
                    fe = work.tile([128, 1024], bf16, tag="fe")
                    xsl = xpv[lv][:, :, p0:p0 + pw]
                    x2f_k = x2f_sb[lv][:].rearrange(
                        "p (h c) -> p h c", h=2)[:, :, p0:p0 + pw]
                    for m in range(2):
                        ov = psF[:, m * 512:(m + 1) * 512].rearrange(
                            "p (h c) -> p h c", h=2)[:, :, 0:pw]
                        ev_ = fe[:, m * 512:(m + 1) * 512].rearrange(
                            "p (h c) -> p h c", h=2)[:, :, 0:pw]
                        mc = slice(m * 128, (m + 1) * 128)
                        if USE_BCAST:
                            xb = xsl.unsqueeze(2).to_broadcast(
                                [128, 2, 2, pw])
                            mm(ov, wv[:, :, mc], xb,
                               start=True, stop=False, dr=True)
                            mm(ov, w2f_sb[:, mc], x2f_k,
                               start=False, stop=False)
                            for k in range(2):
                                hr = hp[:, k, 0:Lc].rearrange(
                                    "p (h c) -> p h c", h=2)[:, :, p0:p0 + pw]
                                mm(ov, uv[:, k, mc], hr,
                                   start=False, stop=(k == 1))
                            nc.scalar.activation(ev_, ov, AF.Sigmoid)
                        else:
                            for hh in range(2):
                                o = psF[:, m * 512 + hh * 256:
                                        m * 512 + hh * 256 + pw]
                                mm(o, wv[:, :, mc], xsl,
                                   start=True, stop=False, dr=True)
                                mm(o, w2f_sb[:, mc],
                                   x2f_k[:, hh, :],
                                   start=False, stop=False)
                                for k in range(2):
                                    mm(o, uv[:, k, mc],
                                       hp[:, k, hh * Lp + p0:
                                          hh * Lp + p0 + pw],
                                       start=False, stop=(k == 1))
                            nc.scalar.activation(ev_, ov, AF.Sigmoid)
                    for m in range(2):
                        fhL = work.tile([128, 256], bf16, tag="fhL")
                        fhR = work.tile([128, 256], bf16, tag="fhR")
                        nc.vector.tensor_mul(
                            fhL[:, 0:pw], fe[:, m * 512:m * 512 + pw],
                            hp[:, m, p0:p0 + pw])
                        nc.vector.tensor_mul(
                            fhR[:, 0:pw],
                            fe[:, m * 512 + 256:m * 512 + 256 + pw],
                            hp[:, m, Lp + p0:Lp + p0 + pw])
                        nc.vector.tensor_add(
                            fsv[:, m, p0:p0 + pw], fhL[:, 0:pw],
                            fhR[:, 0:pw])

                # --- iu, parent chunks ---
                for p0 in range(0, Lp, PCH):
                    pw = min(PCH, Lp - p0)
                    psIU = pa.tile([128, 1024], f32, tag="psA")
                    si = work.tile([128, 512], bf16, tag="si")
                    tu = work.tile([128, 512], bf16, tag="tu")
                    sivw = si[:].rearrange("p (m c) -> p m c", m=2)[
                        :, :, 0:pw]
                    tuvw = tu[:].rearrange("p (m c) -> p m c", m=2)[
                        :, :, 0:pw]
                    piv = psIU[:, 0:512].rearrange(
                        "p (m c) -> p m c", m=2)[:, :, 0:pw]
                    puv = psIU[:, 512:1024].rearrange(
                        "p (m c) -> p m c", m=2)[:, :, 0:pw]
                    for m in range(4):
                        o = psIU[:, m * 256:m * 256 + pw]
                        mc = slice(256 + m * 128, 256 + (m + 1) * 128)
                        mm(o, wv[:, :, mc], xpv[lv][:, :, p0:p0 + pw],
                           start=True, stop=False, dr=True)
                        mm(o, w2iu_sb[:, m * 128:(m + 1) * 128],
                           x2iu_sb[lv][:, p0:p0 + pw],
                           start=False, stop=False)
                        for k in range(2):
                            mm(o, uv[:, k, mc], hsv[:, k, p0:p0 + pw],
                               start=False, stop=(k == 1))
                        if m == 1:
                            nc.scalar.activation(sivw, piv, AF.Sigmoid)
                    nc.scalar.activation(tuvw, puv, AF.Tanh)
                    g = work.tile([128, 512], bf16, tag="g")
                    g2 = work.tile([128, 512], bf16, tag="g2")
                    nc.vector.tensor_mul(g[:, :], si[:, :], tu[:, :])
                    gv = g[:].rearrange("p (m c) -> p m c", m=2)[:, :, 0:pw]
                    g2v = g2[:].rearrange("p (m c) -> p m c", m=2)[:, :, 0:pw]
                    nc.vector.tensor_add(g2v, gv, fsv[:, :, p0:p0 + pw])
                    nc.scalar.activation(hcv[:, :, p0:p0 + pw], g2v, AF.Tanh)

            # ---------------- roots -> output (host transposes) ----------
            nc.sync.dma_start(out=out_d.ap(), in_=h_sb[6][:, :])

    nc.compile()
    return nc


def prep_inputs(tokens, dep, idx2vec, q, W, U, D, b):
    """Host-side prep: per-core input maps with pre-gathered fp8 x streams."""
    import ml_dtypes

    bf = ml_dtypes.bfloat16
    f8 = ml_dtypes.float8_e4m3fn
    tokens = np.asarray(tokens, np.int32)
    dep = np.asarray(dep, np.int32)
    idx2vec = np.asarray(idx2vec, np.float32)
    q = np.asarray(q, np.float32)
    W = np.asarray(W, np.float32)
    U = np.asarray(U, np.float32)
    D = np.asarray(D, np.float32)
    b = np.asarray(b, np.float32)

    emb8 = idx2vec.astype(f8)

    WT = np.ascontiguousarray(W.T)            # [300, 768]
    UT = np.ascontiguousarray(U.T)            # [256, 768]
    qD = q @ D.T                              # [10, 768]
    qdiu = qD[:, 256:] + b[None, 256:] / 2.0  # [10, 512]
    qdf = qD[:, :256] + b[None, :256]         # [10, 256]
    leafconst = qD[-1, 256:] + b[256:]        # [512]

    wk = np.stack([WT[0:128], WT[128:256]])           # [2, 128, 768]
    wk = np.ascontiguousarray(wk.transpose(1, 0, 2)).astype(f8)

    def res8(v):
        a = v.astype(f8).astype(np.float32)
        return a, (v - a)

    w2iu = np.zeros((66, 512), np.float32)
    w2iu[0:44] = WT[256:300, 256:768]
    w2iu[44:54], w2iu[54:64] = res8(qdiu)
    w2iu[64], w2iu[65] = res8(leafconst[None, :])
    w2iu = w2iu.astype(f8)

    w2f = np.zeros((64, 256), np.float32)
    w2f[0:44] = WT[256:300, 0:256]
    w2f[44:54], w2f[54:64] = res8(qdf)
    w2f = w2f.astype(f8)

    uk = np.stack([UT[0:128], UT[128:256]])
    uk = np.ascontiguousarray(uk.transpose(1, 0, 2)).astype(bf)

    shared = dict(wk=wk.reshape(128, -1), w2iu=w2iu,
                  w2f=w2f, uk=uk.reshape(128, -1))

    P = PERM
    pnode = np.maximum((P % 127 - 1) // 2, 0) + (P // 127) * 127  # parent ids
    tt = P // 127
    n = P % 127
    lkid = tt * 127 + 2 * n + 1
    rkid = tt * 127 + 2 * n + 2
    internal = (n < 63)
    isleaf = ~internal

    per_core = []
    for c in range(NCORES):
        tokf = tokens[c * BT:(c + 1) * BT].reshape(-1)
        depf = dep[c * BT:(c + 1) * BT].reshape(-1)
        G8 = emb8[tokf[P]]                    # [NN, 300] fp8
        GP8 = emb8[tokf[pnode]]               # parent rows (for f gates)

        m = dict(shared)
        for lv in range(7):
            s = slice(NOFF[lv], NOFF[lv + 1])
            gs = G8[s]
            xp = np.stack([gs[:, 0:128].T, gs[:, 128:256].T])  # [2,128,L]
            m[f"xp{lv}"] = np.ascontiguousarray(
                xp.transpose(1, 0, 2)).reshape(128, -1)

            a = np.zeros((66, LS[lv]), np.float32)
            a[0:44] = gs[:, 256:300].T.astype(np.float32)
            if lv > 0:
                dl = depf[lkid[s]]
                dr = depf[rkid[s]]
                oh = (dl[None, :] == np.arange(10)[:, None]).astype(
                    np.float32)
                oh += (dr[None, :] == np.arange(10)[:, None])
                a[44:54] = oh
                a[54:64] = oh
            else:
                a[64] = 1.0
                a[65] = 1.0
            m[f"x2iu{lv}"] = np.ascontiguousarray(a.astype(f8))

            if lv > 0:
                sc = slice(NOFF[lv - 1], NOFF[lv])
                gp = GP8[sc]
                af = np.zeros((64, LS[lv - 1]), np.float32)
                af[0:44] = gp[:, 256:300].T.astype(np.float32)
                dc = depf[P[sc]]
                af[44:54] = (dc[None, :] == np.arange(10)[:, None])
                af[54:64] = af[44:54]
                m[f"x2f{lv}"] = np.ascontiguousarray(af.astype(f8))
        per_core.append(m)
    return per_core


_NC_CACHE = {}
TRACE = False
LAST = None


def _get_nc():
    if "nc" not in _NC_CACHE:
        _NC_CACHE["nc"] = build_nc()
    return _NC_CACHE["nc"]


def kernel(tokens, dep, idx2vec, q, W, U, D, b):
    global LAST
    from concourse.bass_utils import run_bass_kernel_spmd

    nc = _get_nc()
    in_maps = prep_inputs(tokens, dep, idx2vec, q, W, U, D, b)
    res = run_bass_kernel_spmd(nc, in_maps, list(range(NCORES)), trace=TRACE)
    LAST = res
    outs = []
    for i in range(NCORES):
        arr = np.asarray(res.results[i]["out"], np.float32)  # [128, 2*BT]
        h = np.empty((BT, 256), np.float32)
        h[:, 0:128] = arr[:, 0:BT].T
        h[:, 128:256] = arr[:, BT:2 * BT].T
        outs.append(h)
    return np.concatenate(outs, axis=0)


# revision 16
# speedup vs baseline: 1.0390x; 1.0390x over previous
"""TreeLSTM-style DERNN kernel for Trainium2 (Bass/Tile), 8-core data-parallel.

Strategy (v2)
-------------
- Shard the 512 trees across 8 cores (64 trees/core); each tree is a
  complete binary tree of 127 nodes, processed level-synchronously
  (leaves -> root).
- Host-side prep does the embedding gather + transpose: x arrives as
  fp8 [feature, node] streams in level-major order, with each level laid
  out [left-children | right-children] so pair reductions are contiguous
  adds and the f-gate can reuse the parent's x stream for both halves.
- All x-side projections run as fp8 e4m3 DoubleRow matmuls (2 K-tiles
  per pass, 0.5 cyc/row). The dep-type terms are folded into the K
  remainder tile (rows 44:54 = one-hot / pair-sum one-hot), the leaf
  iu constant into row 54, and all biases into the host-prepped weight
  tiles, so there are no separate dep matmuls and no ACT biases.
- U·h terms stay bf16 (fp8 h fails accuracy) and accumulate into the
  same PSUM region as the x projections; gates activate directly from
  PSUM. PSUM: 2 pools x 2 bufs x 2 banks = all 8 banks, giving depth-2
  chunk pipelining so the PE never drains (pstate stays at 2.4 GHz).
"""

import os
import sys

import numpy as np

for _p in ("/opt/trn_rl_repo", "/root/.axon_site/_ro/trn_rl_repo"):
    if _p not in sys.path and os.path.isdir(_p):
        sys.path.append(_p)

B, N, H, E, V, Q = 512, 127, 256, 300, 50000, 10
NCORES = 8
BT = B // NCORES          # trees per core
NN = BT * 127             # nodes per core
CN = BT * 126             # child nodes per core (levels 0..5)
LS = [BT * (64 >> lv) for lv in range(7)]    # level sizes, lv0 = leaves
NOFF = [0]
for _lv in range(7):
    NOFF.append(NOFF[-1] + LS[_lv])

PCH = 256    # parent chunk (internal levels)
LCH = 512    # leaf chunk

USE_BCAST = True   # single f-x matmul with broadcast-halves AP


def _order():
    """Level-major node order; within each level [left kids | right kids]
    of the previous (parent) level's order. Returns flat node ids."""
    t = np.arange(BT) * 127
    ords = {6: t.copy()}                     # roots
    for lv in range(5, -1, -1):
        par = ords[lv + 1]
        tt = par // 127
        n = par % 127
        left = tt * 127 + 2 * n + 1
        right = tt * 127 + 2 * n + 2
        ords[lv] = np.concatenate([left, right])
    return np.concatenate([ords[lv] for lv in range(7)])


PERM = _order()


def build_nc():
    import concourse.bacc as bacc
    import concourse.bass as bass  # noqa: F401
    import concourse.mybir as mybir
    import concourse.tile as tile

    f32 = mybir.dt.float32
    bf16 = mybir.dt.bfloat16
    f8 = mybir.dt.float8e4
    AF = mybir.ActivationFunctionType
    DR = mybir.MatmulPerfMode.DoubleRow

    nc = bacc.Bacc("TRN2", target_bir_lowering=False, debug=False,
                   num_devices=NCORES)

    xp_d = [nc.declare_dram_parameter(f"xp{lv}", [128, 2 * LS[lv]], f8,
                                      isOutput=False) for lv in range(7)]
    x2iu_d = [nc.declare_dram_parameter(f"x2iu{lv}", [66, LS[lv]], f8,
                                        isOutput=False) for lv in range(7)]
    x2f_d = [None] + [nc.declare_dram_parameter(
        f"x2f{lv}", [64, LS[lv - 1]], f8, isOutput=False)
        for lv in range(1, 7)]
    w_d = nc.declare_dram_parameter("wk", [128, 2 * 768], f8, isOutput=False)
    w2iu_d = nc.declare_dram_parameter("w2iu", [66, 512], f8,
                                       isOutput=False)
    w2f_d = nc.declare_dram_parameter("w2f", [64, 256], f8,
                                      isOutput=False)
    u_d = nc.declare_dram_parameter("uk", [128, 2 * 768], bf16,
                                    isOutput=False)
    out_d = nc.declare_dram_parameter("out", [128, 2 * BT], bf16,
                                      isOutput=True)

    with tile.TileContext(nc) as tc:
        with (
            tc.tile_pool(name="const", bufs=1) as const,
            tc.tile_pool(name="pa", bufs=2, space="PSUM") as pa,
            tc.tile_pool(name="pb", bufs=2, space="PSUM") as pb,
            tc.tile_pool(name="work", bufs=3) as work,
        ):
            def load(dram, shape, dtype, eng=None):
                t = const.tile(shape, dtype, name=f"ld_{dram.name}")
                (eng or nc.sync).dma_start(out=t[:], in_=dram.ap())
                return t

            w_sb = load(w_d, [128, 2 * 768], f8)
            w2iu_sb = load(w2iu_d, [66, 512], f8)
            w2f_sb = load(w2f_d, [64, 256], f8)
            u_sb = load(u_d, [128, 2 * 768], bf16)

            def load2(dram, shape, dtype, nsplit):
                """Load a [P, k*L] k-tiled tensor in nsplit column slices
                so compute can start before the whole tensor arrives."""
                t = const.tile(shape, dtype, name=f"ld_{dram.name}")
                P_, tot = shape
                L = tot // 2
                step = L // nsplit
                tv = t[:].rearrange("p (k n) -> p k n", k=2)
                dv = dram.ap().rearrange("p (k n) -> p k n", k=2)
                for s0 in range(0, L, step):
                    nc.sync.dma_start(out=tv[:, :, s0:s0 + step],
                                      in_=dv[:, :, s0:s0 + step])
                return t

            def load1(dram, shape, dtype, nsplit):
                t = const.tile(shape, dtype, name=f"ld_{dram.name}")
                P_, L = shape
                step = L // nsplit
                for s0 in range(0, L, step):
                    nc.gpsimd.dma_start(out=t[:, s0:s0 + step],
                                        in_=dram.ap()[:, s0:s0 + step])
                return t

            xp_sb = [None] * 7
            x2iu_sb = [None] * 7
            xp_sb[0] = load2(xp_d[0], [128, 2 * LS[0]], f8, 8)
            x2iu_sb[0] = load1(x2iu_d[0], [66, LS[0]], f8, 8)
            for lv in range(1, 7):
                xp_sb[lv] = load(xp_d[lv], [128, 2 * LS[lv]], f8)
                x2iu_sb[lv] = load(x2iu_d[lv], [66, LS[lv]], f8, nc.gpsimd)
            x2f_sb = [None] + [load(x2f_d[lv], [64, LS[lv - 1]], f8,
                                    nc.gpsimd)
                               for lv in range(1, 7)]

            h_sb = [const.tile([128, 2 * LS[lv]], bf16, name=f"h{lv}")
                    for lv in range(7)]
            hs_sb = [None] + [const.tile([128, 2 * LS[lv]], bf16,
                                         name=f"hs{lv}")
                              for lv in range(1, 7)]
            fs_sb = [None] + [const.tile([128, 2 * LS[lv]], bf16,
                                         name=f"fs{lv}")
                              for lv in range(1, 7)]

            # k-tile views
            wv = w_sb[:].rearrange("p (k m) -> p k m", k=2)       # [128,2,768]
            uv = u_sb[:].rearrange("p (k m) -> p k m", k=2)       # [128,2,768]
            xpv = [xp_sb[lv][:].rearrange("p (k n) -> p k n", k=2)
                   for lv in range(7)]

            def mm(o, lhsT, rhs, start, stop, dr=False):
                nc.tensor.matmul(o, lhsT, rhs, start=start, stop=stop,
                                 perf_mode=DR if dr else None)

            # ---------------- leaves (lv 0) ----------------
            L0 = LS[0]
            for p0 in range(0, L0, LCH):
                cw = min(LCH, L0 - p0)
                psI = pa.tile([128, 1024], f32, tag="psA")
                psU = pb.tile([128, 1024], f32, tag="psB")
                si = work.tile([128, 1024], bf16, tag="siL")
                tu = work.tile([128, 1024], bf16, tag="tuL")
                for m in range(4):
                    ps = psI if m < 2 else psU
                    o = ps[:, (m % 2) * 512:(m % 2) * 512 + cw]
                    mc = slice(256 + m * 128, 256 + (m + 1) * 128)
                    mm(o, wv[:, :, mc], xpv[0][:, :, p0:p0 + cw],
                       start=True, stop=False, dr=True)
                    mm(o, w2iu_sb[:, m * 128:(m + 1) * 128],
                       x2iu_sb[0][:, p0:p0 + cw],
                       start=False, stop=True)
                    s = slice((m % 2) * 512, (m % 2) * 512 + cw)
                    if m == 1:
                        nc.scalar.activation(si[:, 0:512], psI[:, 0:512],
                                             AF.Sigmoid)
                        nc.scalar.activation(si[:, 512:1024],
                                             psI[:, 512:1024], AF.Sigmoid)
                for bk in range(2):
                    s = slice(bk * 512, bk * 512 + cw)
                    nc.scalar.activation(tu[:, s], psU[:, s], AF.Tanh)
                g = work.tile([128, 1024], bf16, tag="gL")
                nc.vector.tensor_mul(g[:, :], si[:, :], tu[:, :])
                gvw = g[:].rearrange("p (m c) -> p m c", m=2)[:, :, 0:cw]
                hovw = h_sb[0][:].rearrange("p (m c) -> p m c", m=2)[
                    :, :, p0:p0 + cw]
                nc.scalar.activation(hovw, gvw, AF.Tanh)

            # ---------------- internal levels ----------------
            for lv in range(1, 7):
                Lp, Lc = LS[lv], LS[lv - 1]
                hp = h_sb[lv - 1][:].rearrange("p (m c) -> p m c", m=2)
                hsv = hs_sb[lv][:].rearrange("p (m c) -> p m c", m=2)
                fsv = fs_sb[lv][:].rearrange("p (m c) -> p m c", m=2)
                hcv = h_sb[lv][:].rearrange("p (m c) -> p m c", m=2)

                # h_sum = h_left + h_right (contiguous halves)
                nc.vector.tensor_add(hsv[:, :, :],
                                     hp[:, :, 0:Lp], hp[:, :, Lp:Lc])

                # --- f gates, parent chunks (left+right kids in one psum) ---
                for p0 in range(0, Lp, PCH):
                    pw = min(PCH, Lp - p0)
                    psF = pb.tile([128, 1024], f32, tag="psB")
                    fe = work.tile([128, 1024], bf16, tag="fe")
                    xsl = xpv[lv][:, :, p0:p0 + pw]
                    x2f_k = x2f_sb[lv][:].rearrange(
                        "p (h c) -> p h c", h=2)[:, :, p0:p0 + pw]
                    for m in range(2):
                        ov = psF[:, m * 512:(m + 1) * 512].rearrange(
                            "p (h c) -> p h c", h=2)[:, :, 0:pw]
                        ev_ = fe[:, m * 512:(m + 1) * 512].rearrange(
                            "p (h c) -> p h c", h=2)[:, :, 0:pw]
                        mc = slice(m * 128, (m + 1) * 128)
                        if USE_BCAST:
                            xb = xsl.unsqueeze(2).to_broadcast(
                                [128, 2, 2, pw])
                            mm(ov, wv[:, :, mc], xb,
                               start=True, stop=False, dr=True)
                            mm(ov, w2f_sb[:, mc], x2f_k,
                               start=False, stop=False)
                            for k in range(2):
                                hr = hp[:, k, 0:Lc].rearrange(
                                    "p (h c) -> p h c", h=2)[:, :, p0:p0 + pw]
                                mm(ov, uv[:, k, mc], hr,
                                   start=False, stop=(k == 1))
                            nc.scalar.activation(ev_, ov, AF.Sigmoid)
                        else:
                            for hh in range(2):
                                o = psF[:, m * 512 + hh * 256:
                                        m * 512 + hh * 256 + pw]
                                mm(o, wv[:, :, mc], xsl,
                                   start=True, stop=False, dr=True)
                                mm(o, w2f_sb[:, mc],
                                   x2f_k[:, hh, :],
                                   start=False, stop=False)
                                for k in range(2):
                                    mm(o, uv[:, k, mc],
                                       hp[:, k, hh * Lp + p0:
                                          hh * Lp + p0 + pw],
                                       start=False, stop=(k == 1))
                            nc.scalar.activation(ev_, ov, AF.Sigmoid)
                    for m in range(2):
                        fhL = work.tile([128, 256], bf16, tag="fhL")
                        fhR = work.tile([128, 256], bf16, tag="fhR")
                        nc.vector.tensor_mul(
                            fhL[:, 0:pw], fe[:, m * 512:m * 512 + pw],
                            hp[:, m, p0:p0 + pw])
                        nc.vector.tensor_mul(
                            fhR[:, 0:pw],
                            fe[:, m * 512 + 256:m * 512 + 256 + pw],
                            hp[:, m, Lp + p0:Lp + p0 + pw])
                        nc.vector.tensor_add(
                            fsv[:, m, p0:p0 + pw], fhL[:, 0:pw],
                            fhR[:, 0:pw])

                # --- iu, parent chunks ---
                for p0 in range(0, Lp, PCH):
                    pw = min(PCH, Lp - p0)
                    psIU = pa.tile([128, 1024], f32, tag="psA")
                    si = work.tile([128, 512], bf16, tag="si")
                    tu = work.tile([128, 512], bf16, tag="tu")
                    sivw = si[:].rearrange("p (m c) -> p m c", m=2)[
                        :, :, 0:pw]
                    tuvw = tu[:].rearrange("p (m c) -> p m c", m=2)[
                        :, :, 0:pw]
                    piv = psIU[:, 0:512].rearrange(
                        "p (m c) -> p m c", m=2)[:, :, 0:pw]
                    puv = psIU[:, 512:1024].rearrange(
                        "p (m c) -> p m c", m=2)[:, :, 0:pw]
                    for m in range(4):
                        o = psIU[:, m * 256:m * 256 + pw]
                        mc = slice(256 + m * 128, 256 + (m + 1) * 128)
                        mm(o, wv[:, :, mc], xpv[lv][:, :, p0:p0 + pw],
                           start=True, stop=False, dr=True)
                        mm(o, w2iu_sb[:, m * 128:(m + 1) * 128],
                           x2iu_sb[lv][:, p0:p0 + pw],
                           start=False, stop=False)
                        for k in range(2):
                            mm(o, uv[:, k, mc], hsv[:, k, p0:p0 + pw],
                               start=False, stop=(k == 1))
                        if m == 1:
                            nc.scalar.activation(sivw, piv, AF.Sigmoid)
                    nc.scalar.activation(tuvw, puv, AF.Tanh)
                    g = work.tile([128, 512], bf16, tag="g")
                    g2 = work.tile([128, 512], bf16, tag="g2")
                    nc.vector.tensor_mul(g[:, :], si[:, :], tu[:, :])
                    gv = g[:].rearrange("p (m c) -> p m c", m=2)[:, :, 0:pw]
                    g2v = g2[:].rearrange("p (m c) -> p m c", m=2)[:, :, 0:pw]
                    nc.vector.tensor_add(g2v, gv, fsv[:, :, p0:p0 + pw])
                    nc.scalar.activation(hcv[:, :, p0:p0 + pw], g2v, AF.Tanh)

            # ---------------- roots -> output (host transposes) ----------
            nc.sync.dma_start(out=out_d.ap(), in_=h_sb[6][:, :])

    nc.compile()
    return nc


def prep_inputs(tokens, dep, idx2vec, q, W, U, D, b):
    """Host-side prep: per-core input maps with pre-gathered fp8 x streams."""
    import ml_dtypes

    bf = ml_dtypes.bfloat16
    f8 = ml_dtypes.float8_e4m3fn
    tokens = np.asarray(tokens, np.int32)
    dep = np.asarray(dep, np.int32)
    idx2vec = np.asarray(idx2vec, np.float32)
    q = np.asarray(q, np.float32)
    W = np.asarray(W, np.float32)
    U = np.asarray(U, np.float32)
    D = np.asarray(D, np.float32)
    b = np.asarray(b, np.float32)

    emb8 = idx2vec.astype(f8)

    WT = np.ascontiguousarray(W.T)            # [300, 768]
    UT = np.ascontiguousarray(U.T)            # [256, 768]
    qD = q @ D.T                              # [10, 768]
    qdiu = qD[:, 256:] + b[None, 256:] / 2.0  # [10, 512]
    qdf = qD[:, :256] + b[None, :256]         # [10, 256]
    leafconst = qD[-1, 256:] + b[256:]        # [512]

    wk = np.stack([WT[0:128], WT[128:256]])           # [2, 128, 768]
    wk = np.ascontiguousarray(wk.transpose(1, 0, 2)).astype(f8)

    def res8(v):
        a = v.astype(f8).astype(np.float32)
        return a, (v - a)

    w2iu = np.zeros((66, 512), np.float32)
    w2iu[0:44] = WT[256:300, 256:768]
    w2iu[44:54], w2iu[54:64] = res8(qdiu)
    w2iu[64], w2iu[65] = res8(leafconst[None, :])
    w2iu = w2iu.astype(f8)

    w2f = np.zeros((64, 256), np.float32)
    w2f[0:44] = WT[256:300, 0:256]
    w2f[44:54], w2f[54:64] = res8(qdf)
    w2f = w2f.astype(f8)

    uk = np.stack([UT[0:128], UT[128:256]])
    uk = np.ascontiguousarray(uk.transpose(1, 0, 2)).astype(bf)

    shared = dict(wk=wk.reshape(128, -1), w2iu=w2iu,
                  w2f=w2f, uk=uk.reshape(128, -1))

    P = PERM
    pnode = np.maximum((P % 127 - 1) // 2, 0) + (P // 127) * 127  # parent ids
    tt = P // 127
    n = P % 127
    lkid = tt * 127 + 2 * n + 1
    rkid = tt * 127 + 2 * n + 2
    internal = (n < 63)
    isleaf = ~internal

    per_core = []
    for c in range(NCORES):
        tokf = tokens[c * BT:(c + 1) * BT].reshape(-1)
        depf = dep[c * BT:(c + 1) * BT].reshape(-1)
        G8 = emb8[tokf[P]]                    # [NN, 300] fp8
        GP8 = emb8[tokf[pnode]]               # parent rows (for f gates)

        m = dict(shared)
        for lv in range(7):
            s = slice(NOFF[lv], NOFF[lv + 1])
            gs = G8[s]
            xp = np.stack([gs[:, 0:128].T, gs[:, 128:256].T])  # [2,128,L]
            m[f"xp{lv}"] = np.ascontiguousarray(
                xp.transpose(1, 0, 2)).reshape(128, -1)

            a = np.zeros((66, LS[lv]), np.float32)
            a[0:44] = gs[:, 256:300].T.astype(np.float32)
            if lv > 0:
                dl = depf[lkid[s]]
                dr = depf[rkid[s]]
                oh = (dl[None, :] == np.arange(10)[:, None]).astype(
                    np.float32)
                oh += (dr[None, :] == np.arange(10)[:, None])
                a[44:54] = oh
                a[54:64] = oh
            else:
                a[64] = 1.0
                a[65] = 1.0
            m[f"x2iu{lv}"] = np.ascontiguousarray(a.astype(f8))

            if lv > 0:
                sc = slice(NOFF[lv - 1], NOFF[lv])
                gp = GP8[sc]
                af = np.zeros((64, LS[lv - 1]), np.float32)
                af[0:44] = gp[:, 256:300].T.astype(np.float32)
                dc = depf[P[sc]]
                af[44:54] = (dc[None, :] == np.arange(10)[:, None])
                af[54:64] = af[44:54]
                m[f"x2f{lv}"] = np.ascontiguousarray(af.astype(f8))
        per_core.append(m)
    return per_core


_NC_CACHE = {}
TRACE = False
LAST = None


def _get_nc():
    if "nc" not in _NC_CACHE:
        _NC_CACHE["nc"] = build_nc()
    return _NC_CACHE["nc"]


def kernel(tokens, dep, idx2vec, q, W, U, D, b):
    global LAST
    from concourse.bass_utils import run_bass_kernel_spmd

    nc = _get_nc()
    in_maps = prep_inputs(tokens, dep, idx2vec, q, W, U, D, b)
    res = run_bass_kernel_spmd(nc, in_maps, list(range(NCORES)), trace=TRACE)
    LAST = res
    outs = []
    for i in range(NCORES):
        arr = np.asarray(res.results[i]["out"], np.float32)  # [128, 2*BT]
        h = np.empty((BT, 256), np.float32)
        h[:, 0:128] = arr[:, 0:BT].T
        h[:, 128:256] = arr[:, BT:2 * BT].T
        outs.append(h)
    return np.concatenate(outs, axis=0)


# revision 17
# speedup vs baseline: 1.1233x; 1.0811x over previous
"""TreeLSTM-style DERNN kernel for Trainium2 (Bass/Tile), 8-core data-parallel.

Strategy (v2)
-------------
- Shard the 512 trees across 8 cores (64 trees/core); each tree is a
  complete binary tree of 127 nodes, processed level-synchronously
  (leaves -> root).
- Host-side prep does the embedding gather + transpose: x arrives as
  fp8 [feature, node] streams in level-major order, with each level laid
  out [left-children | right-children] so pair reductions are contiguous
  adds and the f-gate can reuse the parent's x stream for both halves.
- All x-side projections run as fp8 e4m3 DoubleRow matmuls (2 K-tiles
  per pass, 0.5 cyc/row). The dep-type terms are folded into the K
  remainder tile (rows 44:54 = one-hot / pair-sum one-hot), the leaf
  iu constant into row 54, and all biases into the host-prepped weight
  tiles, so there are no separate dep matmuls and no ACT biases.
- U·h terms stay bf16 (fp8 h fails accuracy) and accumulate into the
  same PSUM region as the x projections; gates activate directly from
  PSUM. PSUM: 2 pools x 2 bufs x 2 banks = all 8 banks, giving depth-2
  chunk pipelining so the PE never drains (pstate stays at 2.4 GHz).
"""

import os
import sys

import numpy as np

for _p in ("/opt/trn_rl_repo", "/root/.axon_site/_ro/trn_rl_repo"):
    if _p not in sys.path and os.path.isdir(_p):
        sys.path.append(_p)

B, N, H, E, V, Q = 512, 127, 256, 300, 50000, 10
NCORES = 8
BT = B // NCORES          # trees per core
NN = BT * 127             # nodes per core
CN = BT * 126             # child nodes per core (levels 0..5)
LS = [BT * (64 >> lv) for lv in range(7)]    # level sizes, lv0 = leaves
NOFF = [0]
for _lv in range(7):
    NOFF.append(NOFF[-1] + LS[_lv])

PCH = 256    # parent chunk (internal levels)
LCH = 512    # leaf chunk

USE_BCAST = True   # single f-x matmul with broadcast-halves AP


def _order():
    """Level-major node order; within each level [left kids | right kids]
    of the previous (parent) level's order. Returns flat node ids."""
    t = np.arange(BT) * 127
    ords = {6: t.copy()}                     # roots
    for lv in range(5, -1, -1):
        par = ords[lv + 1]
        tt = par // 127
        n = par % 127
        left = tt * 127 + 2 * n + 1
        right = tt * 127 + 2 * n + 2
        ords[lv] = np.concatenate([left, right])
    return np.concatenate([ords[lv] for lv in range(7)])


PERM = _order()


def build_nc():
    import concourse.bacc as bacc
    import concourse.bass as bass  # noqa: F401
    import concourse.mybir as mybir
    import concourse.tile as tile

    f32 = mybir.dt.float32
    bf16 = mybir.dt.bfloat16
    f8 = mybir.dt.float8e4
    AF = mybir.ActivationFunctionType
    DR = mybir.MatmulPerfMode.DoubleRow

    nc = bacc.Bacc("TRN2", target_bir_lowering=False, debug=False,
                   num_devices=NCORES)

    xp_d = [nc.declare_dram_parameter(f"xp{lv}", [128, 2 * LS[lv]], f8,
                                      isOutput=False) for lv in range(7)]
    x2iu_d = [nc.declare_dram_parameter(f"x2iu{lv}", [66, LS[lv]], f8,
                                        isOutput=False) for lv in range(7)]
    x2f_d = [None] + [nc.declare_dram_parameter(
        f"x2f{lv}", [64, LS[lv - 1]], f8, isOutput=False)
        for lv in range(1, 7)]
    w_d = nc.declare_dram_parameter("wk", [128, 2 * 768], f8, isOutput=False)
    w2iu_d = nc.declare_dram_parameter("w2iu", [66, 512], f8,
                                       isOutput=False)
    w2f_d = nc.declare_dram_parameter("w2f", [64, 256], f8,
                                      isOutput=False)
    u_d = nc.declare_dram_parameter("uk", [128, 2 * 768], bf16,
                                    isOutput=False)
    out_d = nc.declare_dram_parameter("out", [128, 2 * BT], bf16,
                                      isOutput=True)

    with tile.TileContext(nc) as tc:
        with (
            tc.tile_pool(name="const", bufs=1) as const,
            tc.tile_pool(name="pa", bufs=2, space="PSUM") as pa,
            tc.tile_pool(name="pb", bufs=2, space="PSUM") as pb,
            tc.tile_pool(name="work", bufs=3) as work,
        ):
            def load(dram, shape, dtype, eng=None):
                t = const.tile(shape, dtype, name=f"ld_{dram.name}")
                (eng or nc.sync).dma_start(out=t[:], in_=dram.ap())
                return t

            w_sb = load(w_d, [128, 2 * 768], f8)
            w2iu_sb = load(w2iu_d, [66, 512], f8)
            w2f_sb = load(w2f_d, [64, 256], f8)
            u_sb = load(u_d, [128, 2 * 768], bf16)

            def load2(dram, shape, dtype, nsplit):
                """Load a [P, k*L] k-tiled tensor in nsplit column slices
                so compute can start before the whole tensor arrives."""
                t = const.tile(shape, dtype, name=f"ld_{dram.name}")
                P_, tot = shape
                L = tot // 2
                step = L // nsplit
                tv = t[:].rearrange("p (k n) -> p k n", k=2)
                dv = dram.ap().rearrange("p (k n) -> p k n", k=2)
                for s0 in range(0, L, step):
                    nc.sync.dma_start(out=tv[:, :, s0:s0 + step],
                                      in_=dv[:, :, s0:s0 + step])
                return t

            def load1(dram, shape, dtype, nsplit):
                t = const.tile(shape, dtype, name=f"ld_{dram.name}")
                P_, L = shape
                step = L // nsplit
                for s0 in range(0, L, step):
                    nc.gpsimd.dma_start(out=t[:, s0:s0 + step],
                                        in_=dram.ap()[:, s0:s0 + step])
                return t

            xp_sb = [None] * 7
            x2iu_sb = [None] * 7
            xp_sb[0] = load2(xp_d[0], [128, 2 * LS[0]], f8, 8)
            x2iu_sb[0] = load1(x2iu_d[0], [66, LS[0]], f8, 8)
            for lv in range(1, 7):
                xp_sb[lv] = load(xp_d[lv], [128, 2 * LS[lv]], f8)
                x2iu_sb[lv] = load(x2iu_d[lv], [66, LS[lv]], f8, nc.gpsimd)
            x2f_sb = [None] + [load(x2f_d[lv], [64, LS[lv - 1]], f8,
                                    nc.gpsimd)
                               for lv in range(1, 7)]

            h_sb = [const.tile([128, 2 * LS[lv]], bf16, name=f"h{lv}")
                    for lv in range(7)]
            hs_sb = [None] + [const.tile([128, 2 * LS[lv]], bf16,
                                         name=f"hs{lv}")
                              for lv in range(1, 7)]
            fs_sb = [None] + [const.tile([128, 2 * LS[lv]], bf16,
                                         name=f"fs{lv}")
                              for lv in range(1, 7)]

            # k-tile views
            wv = w_sb[:].rearrange("p (k m) -> p k m", k=2)       # [128,2,768]
            uv = u_sb[:].rearrange("p (k m) -> p k m", k=2)       # [128,2,768]
            xpv = [xp_sb[lv][:].rearrange("p (k n) -> p k n", k=2)
                   for lv in range(7)]

            def mm(o, lhsT, rhs, start, stop, dr=False):
                nc.tensor.matmul(o, lhsT, rhs, start=start, stop=stop,
                                 perf_mode=DR if dr else None)

            # ---------------- leaves (lv 0) ----------------
            L0 = LS[0]
            for p0 in range(0, L0, LCH):
                cw = min(LCH, L0 - p0)
                psI = pa.tile([128, 1024], f32, tag="psA")
                psU = pb.tile([128, 1024], f32, tag="psB")
                si = work.tile([128, 1024], bf16, tag="siL")
                tu = work.tile([128, 1024], bf16, tag="tuL")
                for m in range(4):
                    ps = psI if m < 2 else psU
                    o = ps[:, (m % 2) * 512:(m % 2) * 512 + cw]
                    mc = slice(256 + m * 128, 256 + (m + 1) * 128)
                    mm(o, wv[:, :, mc], xpv[0][:, :, p0:p0 + cw],
                       start=True, stop=False, dr=True)
                    mm(o, w2iu_sb[:, m * 128:(m + 1) * 128],
                       x2iu_sb[0][:, p0:p0 + cw],
                       start=False, stop=True)
                for bk in range(2):
                    s = slice(bk * 512, bk * 512 + cw)
                    nc.scalar.activation(si[:, s], psI[:, s], AF.Sigmoid)
                    nc.scalar.activation(tu[:, s], psU[:, s], AF.Tanh)
                g = work.tile([128, 1024], bf16, tag="gL")
                nc.vector.tensor_mul(g[:, :], si[:, :], tu[:, :])
                gvw = g[:].rearrange("p (m c) -> p m c", m=2)[:, :, 0:cw]
                hovw = h_sb[0][:].rearrange("p (m c) -> p m c", m=2)[
                    :, :, p0:p0 + cw]
                nc.scalar.activation(hovw, gvw, AF.Tanh)

            # ---------------- internal levels ----------------
            for lv in range(1, 7):
                Lp, Lc = LS[lv], LS[lv - 1]
                hp = h_sb[lv - 1][:].rearrange("p (m c) -> p m c", m=2)
                hsv = hs_sb[lv][:].rearrange("p (m c) -> p m c", m=2)
                fsv = fs_sb[lv][:].rearrange("p (m c) -> p m c", m=2)
                hcv = h_sb[lv][:].rearrange("p (m c) -> p m c", m=2)

                # h_sum = h_left + h_right (contiguous halves)
                nc.vector.tensor_add(hsv[:, :, :],
                                     hp[:, :, 0:Lp], hp[:, :, Lp:Lc])

                # --- f gates, parent chunks (left+right kids in one psum) ---
                for p0 in range(0, Lp, PCH):
                    pw = min(PCH, Lp - p0)
                    psF = pb.tile([128, 1024], f32, tag="psB")
                    fe = work.tile([128, 1024], bf16, tag="fe")
                    xsl = xpv[lv][:, :, p0:p0 + pw]
                    x2f_k = x2f_sb[lv][:].rearrange(
                        "p (h c) -> p h c", h=2)[:, :, p0:p0 + pw]
                    for m in range(2):
                        ov = psF[:, m * 512:(m + 1) * 512].rearrange(
                            "p (h c) -> p h c", h=2)[:, :, 0:pw]
                        ev_ = fe[:, m * 512:(m + 1) * 512].rearrange(
                            "p (h c) -> p h c", h=2)[:, :, 0:pw]
                        mc = slice(m * 128, (m + 1) * 128)
                        if USE_BCAST:
                            xb = xsl.unsqueeze(2).to_broadcast(
                                [128, 2, 2, pw])
                            mm(ov, wv[:, :, mc], xb,
                               start=True, stop=False, dr=True)
                            mm(ov, w2f_sb[:, mc], x2f_k,
                               start=False, stop=False)
                            for k in range(2):
                                hr = hp[:, k, 0:Lc].rearrange(
                                    "p (h c) -> p h c", h=2)[:, :, p0:p0 + pw]
                                mm(ov, uv[:, k, mc], hr,
                                   start=False, stop=(k == 1))
                        else:
                            for hh in range(2):
                                o = psF[:, m * 512 + hh * 256:
                                        m * 512 + hh * 256 + pw]
                                mm(o, wv[:, :, mc], xsl,
                                   start=True, stop=False, dr=True)
                                mm(o, w2f_sb[:, mc],
                                   x2f_k[:, hh, :],
                                   start=False, stop=False)
                                for k in range(2):
                                    mm(o, uv[:, k, mc],
                                       hp[:, k, hh * Lp + p0:
                                          hh * Lp + p0 + pw],
                                       start=False, stop=(k == 1))
                    for m in range(2):
                        iv_ = psF[:, m * 512:(m + 1) * 512].rearrange(
                            "p (h c) -> p h c", h=2)[:, :, 0:pw]
                        ev2 = fe[:, m * 512:(m + 1) * 512].rearrange(
                            "p (h c) -> p h c", h=2)[:, :, 0:pw]
                        nc.scalar.activation(ev2, iv_, AF.Sigmoid)
                    for m in range(2):
                        fhL = work.tile([128, 256], bf16, tag="fhL")
                        fhR = work.tile([128, 256], bf16, tag="fhR")
                        nc.vector.tensor_mul(
                            fhL[:, 0:pw], fe[:, m * 512:m * 512 + pw],
                            hp[:, m, p0:p0 + pw])
                        nc.vector.tensor_mul(
                            fhR[:, 0:pw],
                            fe[:, m * 512 + 256:m * 512 + 256 + pw],
                            hp[:, m, Lp + p0:Lp + p0 + pw])
                        nc.vector.tensor_add(
                            fsv[:, m, p0:p0 + pw], fhL[:, 0:pw],
                            fhR[:, 0:pw])

                # --- iu, parent chunks ---
                for p0 in range(0, Lp, PCH):
                    pw = min(PCH, Lp - p0)
                    psIU = pa.tile([128, 1024], f32, tag="psA")
                    si = work.tile([128, 512], bf16, tag="si")
                    tu = work.tile([128, 512], bf16, tag="tu")
                    sivw = si[:].rearrange("p (m c) -> p m c", m=2)[
                        :, :, 0:pw]
                    tuvw = tu[:].rearrange("p (m c) -> p m c", m=2)[
                        :, :, 0:pw]
                    piv = psIU[:, 0:512].rearrange(
                        "p (m c) -> p m c", m=2)[:, :, 0:pw]
                    puv = psIU[:, 512:1024].rearrange(
                        "p (m c) -> p m c", m=2)[:, :, 0:pw]
                    for m in range(4):
                        o = psIU[:, m * 256:m * 256 + pw]
                        mc = slice(256 + m * 128, 256 + (m + 1) * 128)
                        mm(o, wv[:, :, mc], xpv[lv][:, :, p0:p0 + pw],
                           start=True, stop=False, dr=True)
                        mm(o, w2iu_sb[:, m * 128:(m + 1) * 128],
                           x2iu_sb[lv][:, p0:p0 + pw],
                           start=False, stop=False)
                        for k in range(2):
                            mm(o, uv[:, k, mc], hsv[:, k, p0:p0 + pw],
                               start=False, stop=(k == 1))
                    nc.scalar.activation(sivw, piv, AF.Sigmoid)
                    nc.scalar.activation(tuvw, puv, AF.Tanh)
                    g = work.tile([128, 512], bf16, tag="g")
                    g2 = work.tile([128, 512], bf16, tag="g2")
                    nc.vector.tensor_mul(g[:, :], si[:, :], tu[:, :])
                    gv = g[:].rearrange("p (m c) -> p m c", m=2)[:, :, 0:pw]
                    g2v = g2[:].rearrange("p (m c) -> p m c", m=2)[:, :, 0:pw]
                    nc.vector.tensor_add(g2v, gv, fsv[:, :, p0:p0 + pw])
                    nc.scalar.activation(hcv[:, :, p0:p0 + pw], g2v, AF.Tanh)

            # ---------------- roots -> output (host transposes) ----------
            nc.sync.dma_start(out=out_d.ap(), in_=h_sb[6][:, :])

    nc.compile()
    return nc


def prep_inputs(tokens, dep, idx2vec, q, W, U, D, b):
    """Host-side prep: per-core input maps with pre-gathered fp8 x streams."""
    import ml_dtypes

    bf = ml_dtypes.bfloat16
    f8 = ml_dtypes.float8_e4m3fn
    tokens = np.asarray(tokens, np.int32)
    dep = np.asarray(dep, np.int32)
    idx2vec = np.asarray(idx2vec, np.float32)
    q = np.asarray(q, np.float32)
    W = np.asarray(W, np.float32)
    U = np.asarray(U, np.float32)
    D = np.asarray(D, np.float32)
    b = np.asarray(b, np.float32)

    emb8 = idx2vec.astype(f8)

    WT = np.ascontiguousarray(W.T)            # [300, 768]
    UT = np.ascontiguousarray(U.T)            # [256, 768]
    qD = q @ D.T                              # [10, 768]
    qdiu = qD[:, 256:] + b[None, 256:] / 2.0  # [10, 512]
    qdf = qD[:, :256] + b[None, :256]         # [10, 256]
    leafconst = qD[-1, 256:] + b[256:]        # [512]

    wk = np.stack([WT[0:128], WT[128:256]])           # [2, 128, 768]
    wk = np.ascontiguousarray(wk.transpose(1, 0, 2)).astype(f8)

    def res8(v):
        a = v.astype(f8).astype(np.float32)
        return a, (v - a)

    w2iu = np.zeros((66, 512), np.float32)
    w2iu[0:44] = WT[256:300, 256:768]
    w2iu[44:54], w2iu[54:64] = res8(qdiu)
    w2iu[64], w2iu[65] = res8(leafconst[None, :])
    w2iu = w2iu.astype(f8)

    w2f = np.zeros((64, 256), np.float32)
    w2f[0:44] = WT[256:300, 0:256]
    w2f[44:54], w2f[54:64] = res8(qdf)
    w2f = w2f.astype(f8)

    uk = np.stack([UT[0:128], UT[128:256]])
    uk = np.ascontiguousarray(uk.transpose(1, 0, 2)).astype(bf)

    shared = dict(wk=wk.reshape(128, -1), w2iu=w2iu,
                  w2f=w2f, uk=uk.reshape(128, -1))

    P = PERM
    pnode = np.maximum((P % 127 - 1) // 2, 0) + (P // 127) * 127  # parent ids
    tt = P // 127
    n = P % 127
    lkid = tt * 127 + 2 * n + 1
    rkid = tt * 127 + 2 * n + 2
    internal = (n < 63)
    isleaf = ~internal

    per_core = []
    for c in range(NCORES):
        tokf = tokens[c * BT:(c + 1) * BT].reshape(-1)
        depf = dep[c * BT:(c + 1) * BT].reshape(-1)
        G8 = emb8[tokf[P]]                    # [NN, 300] fp8
        GP8 = emb8[tokf[pnode]]               # parent rows (for f gates)

        m = dict(shared)
        for lv in range(7):
            s = slice(NOFF[lv], NOFF[lv + 1])
            gs = G8[s]
            xp = np.stack([gs[:, 0:128].T, gs[:, 128:256].T])  # [2,128,L]
            m[f"xp{lv}"] = np.ascontiguousarray(
                xp.transpose(1, 0, 2)).reshape(128, -1)

            a = np.zeros((66, LS[lv]), np.float32)
            a[0:44] = gs[:, 256:300].T.astype(np.float32)
            if lv > 0:
                dl = depf[lkid[s]]
                dr = depf[rkid[s]]
                oh = (dl[None, :] == np.arange(10)[:, None]).astype(
                    np.float32)
                oh += (dr[None, :] == np.arange(10)[:, None])
                a[44:54] = oh
                a[54:64] = oh
            else:
                a[64] = 1.0
                a[65] = 1.0
            m[f"x2iu{lv}"] = np.ascontiguousarray(a.astype(f8))

            if lv > 0:
                sc = slice(NOFF[lv - 1], NOFF[lv])
                gp = GP8[sc]
                af = np.zeros((64, LS[lv - 1]), np.float32)
                af[0:44] = gp[:, 256:300].T.astype(np.float32)
                dc = depf[P[sc]]
                af[44:54] = (dc[None, :] == np.arange(10)[:, None])
                af[54:64] = af[44:54]
                m[f"x2f{lv}"] = np.ascontiguousarray(af.astype(f8))
        per_core.append(m)
    return per_core


_NC_CACHE = {}
TRACE = False
LAST = None


def _get_nc():
    if "nc" not in _NC_CACHE:
        _NC_CACHE["nc"] = build_nc()
    return _NC_CACHE["nc"]


def kernel(tokens, dep, idx2vec, q, W, U, D, b):
    global LAST
    from concourse.bass_utils import run_bass_kernel_spmd

    nc = _get_nc()
    in_maps = prep_inputs(tokens, dep, idx2vec, q, W, U, D, b)
    res = run_bass_kernel_spmd(nc, in_maps, list(range(NCORES)), trace=TRACE)
    LAST = res
    outs = []
    for i in range(NCORES):
        arr = np.asarray(res.results[i]["out"], np.float32)  # [128, 2*BT]
        h = np.empty((BT, 256), np.float32)
        h[:, 0:128] = arr[:, 0:BT].T
        h[:, 128:256] = arr[:, BT:2 * BT].T
        outs.append(h)
    return np.concatenate(outs, axis=0)
